# revision 1
# baseline (speedup 1.0000x reference)
"""Trainium2 Bass kernel for nn_RelPosRFFBias.

Computes bias[b, h, t, s] = MLP(RFF(|c[b,t] - c[b,s]|)) with a tiny
per-pair MLP (32 -> 64 -> 16), data-parallel over B across 8 NeuronCores.

Per-core dataflow (B=1 slice, T=512), per 4-row group / 48-row chunk:
  - DVE : x = (c_s - c_t) * f_k           (per group, per-partition scalars)
  - GPS : x = |x|  (bitwise-and sign clear, chunk-wide, in place)
  - GPS : u = (x + phi_k/2pi) + MAGIC     (chunk-wide; MAGIC rints m to u)
  - DVE : w = (u - MAGIC) - x             (= rint(m) - x, chunk slices)
  - ACT : feats = sin(-2pi*w + phi_k)     (sin/cos RFF features: partition
          layout duplicates each 16-freq block with a +pi/2 phase; HW Sin
          needs post-affine args in [-pi, pi]; boundary spill ~5e-5 clamps)
  - PE  : mm1 block-diag [64,128] W1 (2 t-rows/stream), gelu(+b1) on ACT,
          mm2 block-diag [128,32] W2, 4 results packed into one PSUM bank.
  - ACT : +b2 PSUM->SBUF (Identity, in every act table set), then DMA out
          [8 t-rows, 16 heads, 512 s] per store. DVE runs only prep ops so
          chunk k+1 prep never queues behind chunk k MLP results.
"""

import math

import numpy as np

B, T = 8, 512
RFF, HID, NH = 16, 64, 16
F_MIN, F_MAX = 2.0, 64.0
TWO_PI = 2.0 * math.pi
MAGIC = 12582912.0  # 1.5 * 2^23: forces fp32 round-to-nearest-integer
# largest fp32 <= 2*pi, so |w|<=0.5 maps inside the HW Sin domain [-pi, pi]
TWO_PI_SAFE = 6.283185005187988

N_CORES = 8
CHUNK_ROWS = 64          # t-rows per ACT table phase (sin phase / gelu phase)
GROUPS = T // 4          # 4-row feature groups total
SIN_GROUPS = 4           # feature groups per Sin activation op

_MODULE = None
_LAST_RESULTS = None


def _build_module():
    import concourse.bass as bass
    import concourse.tile as tile
    from concourse import bacc, mybir
    from concourse.tile import add_dep_helper
    from contextlib import ExitStack

    f32 = mybir.dt.float32
    u32 = mybir.dt.uint32
    Alu = mybir.AluOpType
    Act = mybir.ActivationFunctionType

    nc = bacc.Bacc("TRN2", target_bir_lowering=False, debug=False)

    cb_d = nc.dram_tensor("cb", [128, T], f32, kind="ExternalInput")
    ccol_d = nc.dram_tensor("ccol", [128, GROUPS], f32, kind="ExternalInput")
    # cols: 0=fcol, 1=phicol (phi/2pi + .25*cosblock), 2=b1col, 3=b2col,
    #       4=phi2pi (phi + pi/2*cosblock)
    cols_d = nc.dram_tensor("cols", [128, 5], f32, kind="ExternalInput")
    w1_d = nc.dram_tensor("w1bd", [128, 128], f32, kind="ExternalInput")
    w2_d = nc.dram_tensor("w2bd", [128, 32], f32, kind="ExternalInput")
    out_d = nc.dram_tensor("out", [NH, T, T], f32, kind="ExternalOutput")

    chunks = []
    r = 0
    while r < T:
        chunks.append((r, min(CHUNK_ROWS, T - r)))
        r += CHUNK_ROWS

    with tile.TileContext(nc) as tc:
        with ExitStack() as ctx:
            const = ctx.enter_context(tc.tile_pool(name="const", bufs=1))
            xpool = ctx.enter_context(tc.tile_pool(name="x", bufs=1))
            upool = ctx.enter_context(tc.tile_pool(name="u", bufs=1))
            gpool = ctx.enter_context(tc.tile_pool(name="g", bufs=1))
            fpool = ctx.enter_context(tc.tile_pool(name="feats", bufs=2))
            hpool = ctx.enter_context(tc.tile_pool(name="h", bufs=4))
            opool = ctx.enter_context(tc.tile_pool(name="ostage", bufs=8))
            p1pool = ctx.enter_context(
                tc.tile_pool(name="p1", bufs=3, space="PSUM")
            )
            p2pool = ctx.enter_context(
                tc.tile_pool(name="p2", bufs=2, space="PSUM")
            )

            cb_t = const.tile([128, T], f32)
            nc.sync.dma_start(cb_t[:], cb_d.ap())
            ccol_t = const.tile([128, GROUPS], f32)
            nc.sync.dma_start(ccol_t[:], ccol_d.ap())
            cols_t = const.tile([128, 5], f32)
            nc.sync.dma_start(cols_t[:], cols_d.ap())
            w1_t = const.tile([128, 128], f32)
            nc.sync.dma_start(w1_t[:], w1_d.ap())
            w2_t = const.tile([128, 32], f32)
            nc.sync.dma_start(w2_t[:], w2_d.ap())

            out_ap = out_d.ap().rearrange("h t s -> t h s")  # [T, NH, T]

            prev_gelu = None
            for t0, rows in chunks:
                ngroup = rows // 4
                cw = ngroup * T

                # ---- prep phase: range-reduced sin arguments
                x_t = xpool.tile([128, CHUNK_ROWS // 4 * T], f32, tag="x")
                for gi in range(ngroup):
                    gcol = t0 // 4 + gi
                    nc.vector.tensor_scalar(
                        x_t[:, gi * T : (gi + 1) * T], cb_t[:],
                        ccol_t[:, gcol : gcol + 1], cols_t[:, 0:1],
                        Alu.subtract, Alu.mult,
                    )
                # |x|: clear sign bits, chunk-wide
                nc.vector.tensor_scalar(
                    x_t[:, 0:cw].bitcast(u32), x_t[:, 0:cw].bitcast(u32),
                    0x7FFFFFFF, None, Alu.bitwise_and,
                )
                # u = (x + phicol) + MAGIC, chunk-wide
                u_t = upool.tile([128, CHUNK_ROWS // 4 * T], f32, tag="u")
                nc.vector.tensor_scalar(
                    u_t[:, 0:cw], x_t[:, 0:cw], cols_t[:, 1:2], MAGIC,
                    Alu.add, Alu.add,
                )
                # w = (u - MAGIC) - x, per sin-piece on DVE
                g_t = gpool.tile([128, CHUNK_ROWS // 4 * T], f32, tag="g")
                pieces = []
                for p0 in range(0, ngroup, SIN_GROUPS):
                    pc = slice(p0 * T, min(p0 + SIN_GROUPS, ngroup) * T)
                    nc.vector.scalar_tensor_tensor(
                        g_t[:, pc], u_t[:, pc], MAGIC, x_t[:, pc],
                        Alu.subtract, Alu.subtract,
                    )
                    pieces.append(pc)

                # ---- ACT sin phase (trig table)
                f_t = fpool.tile([128, CHUNK_ROWS // 4 * T], f32, tag="f")
                sin_last = None
                for pc in pieces:
                    sin_last = nc.scalar.activation(
                        f_t[:, pc], g_t[:, pc], Act.Sin,
                        bias=cols_t[:, 4:5], scale=-TWO_PI_SAFE,
                    )
                    if prev_gelu is not None:
                        add_dep_helper(
                            sin_last.ins, prev_gelu.ins, sync=False,
                            reason="ACT table phase: sins after prev gelus",
                        )

                # ---- MLP phase (gelu table)
                pending = None  # (p2_t, tr) awaiting +b2 and store
                for j in range(rows // 8):  # 8-row output blocks
                    p2_t = p2pool.tile([128, T], f32)
                    p1s, hs = [], []
                    for q in range(2):  # 4-row group within the block
                        gi = 2 * j + q
                        fc = slice(gi * T, (gi + 1) * T)
                        p1_t = p1pool.tile([128, 2 * T], f32)
                        nc.tensor.matmul(
                            p1_t[:, 0:T], w1_t[0:64, :], f_t[0:64, fc],
                            start=True, stop=True,
                        )
                        nc.tensor.matmul(
                            p1_t[:, T : 2 * T], w1_t[64:128, :],
                            f_t[64:128, fc], start=True, stop=True,
                        )
                        p1s.append(p1_t)
                    for q in range(2):
                        h_t = hpool.tile([128, 2 * T], f32)
                        gelu = nc.scalar.activation(
                            h_t[:], p1s[q][:], Act.Gelu,
                            bias=cols_t[:, 2:3], scale=1.0,
                        )
                        add_dep_helper(
                            gelu.ins, sin_last.ins, sync=False,
                            reason="ACT table phase: gelus after chunk sins",
                        )
                        prev_gelu = gelu
                        hs.append(h_t)
                    for q in range(2):
                        nc.tensor.matmul(
                            p2_t[64 * q : 64 * q + 32, :], w2_t[:],
                            hs[q][:, 0:T], start=True, stop=True,
                            tile_position=(0, 64 * q),
                        )
                        nc.tensor.matmul(
                            p2_t[64 * q + 32 : 64 * q + 64, :], w2_t[:],
                            hs[q][:, T : 2 * T], start=True, stop=True,
                            tile_position=(0, 64 * q + 32),
                        )
                    if pending is not None:
                        pp2, ptr = pending
                        o_t = opool.tile([128, T], f32)
                        prev_gelu = nc.scalar.activation(
                            o_t[:], pp2[:], Act.Identity,
                            bias=cols_t[:, 3:4], scale=1.0,
                        )
                        nc.sync.dma_start(out_ap[ptr : ptr + 8], o_t[:])
                    pending = (p2_t, t0 + j * 8)
                # flush the last block of the chunk
                pp2, ptr = pending
                o_t = opool.tile([128, T], f32)
                prev_gelu = nc.scalar.activation(
                    o_t[:], pp2[:], Act.Identity,
                    bias=cols_t[:, 3:4], scale=1.0,
                )
                nc.sync.dma_start(out_ap[ptr : ptr + 8], o_t[:])

    nc.compile()
    return nc


def _host_prep(c):
    """Per-core input tensors from this core's centers row c [T] f32."""
    p = np.arange(128)
    cb = np.tile(np.asarray(c, np.float32)[None, :], (128, 1))
    g = np.arange(GROUPS)
    ccol = np.asarray(c, np.float32)[4 * g[None, :] + (p // 32)[:, None]]
    return cb.astype(np.float32), np.ascontiguousarray(ccol, np.float32)


def _host_cols(bias_phase, b1, b2):
    p = np.arange(128)
    freqs = np.logspace(
        np.log10(F_MIN), np.log10(F_MAX), RFF, dtype=np.float32
    ).astype(np.float32)
    cols = np.zeros((128, 5), np.float32)
    cols[:, 0] = freqs[p % 16]
    cols[:, 1] = (bias_phase[p % 16] / TWO_PI + 0.25 * ((p // 16) % 2)).astype(
        np.float32
    )
    cols[:, 2] = b1[p % 64]
    cols[:, 3] = b2[p % 16]
    cols[:, 4] = (bias_phase[p % 16] + (math.pi / 2) * ((p // 16) % 2)).astype(
        np.float32
    )
    return cols


def kernel(centers01, mask, bias_phase, W1, b1, W2, b2):
    global _MODULE, _LAST_RESULTS
    from concourse.bass_utils import run_bass_kernel_spmd

    if _MODULE is None:
        _MODULE = _build_module()
    nc = _MODULE

    centers01 = np.asarray(centers01, np.float32)
    bias_phase = np.asarray(bias_phase, np.float32)
    W1 = np.asarray(W1, np.float32)
    b1 = np.asarray(b1, np.float32)
    W2 = np.asarray(W2, np.float32)
    b2 = np.asarray(b2, np.float32)

    cols = _host_cols(bias_phase, b1, b2)
    w1bd = np.zeros((128, 128), np.float32)
    w1bd[0:32, 0:64] = W1
    w1bd[32:64, 64:128] = W1
    w1bd[64:128, :] = w1bd[0:64, :]
    w2bd = np.zeros((128, 32), np.float32)
    w2bd[0:64, 0:16] = W2
    w2bd[64:128, 16:32] = W2

    in_maps = []
    for b in range(N_CORES):
        cb, ccol = _host_prep(centers01[b])
        in_maps.append(
            {"cb": cb, "ccol": ccol, "cols": cols, "w1bd": w1bd, "w2bd": w2bd}
        )

    res = run_bass_kernel_spmd(nc, in_maps, list(range(N_CORES)))
    _LAST_RESULTS = res
    out = np.stack([res.results[b]["out"] for b in range(N_CORES)], axis=0)

    m = np.asarray(mask, bool)
    out = out * (m[:, None, :, None] & m[:, None, None, :]).astype(np.float32)
    return out.astype(np.float32)

